# revision 25
# baseline (speedup 1.0000x reference)
"""Chamfer distance (nn_ChamferDistance) Trainium2 Bass kernel.

Computes, for xyz1/xyz2 of shape (4, 8192, 3) fp32:
    dist[n, m] = |p_n|^2 + |q_m|^2 - 2 p_n.q_m   (per batch)
    dist1 = min over m, dist2 = min over n
Returns (dist1, dist2), each (4, 8192) fp32 — same as the reference.

Strategy (single-pass, negated):
  - The pairwise-distance matrix is produced directly by the TensorEngine via
    an augmented inner product: u_a . v_b = sq(P)[a] + sq(Q)[b] - 2 P_a.Q_b.
    All factors are split into 3 bf16 planes (hi/lo/lolo) so every product the
    PE forms is exact in fp32; dropped cross terms are ~2^-26 relative.  The
    L-side planes are negated on the host so the device computes -dist and
    every reduction becomes a MAX.  Host negates the outputs back.
  - Sharding: 8 cores = 4 batches x 2 halves of N.  Each core computes its
    4096 x 8192 block of -dist ONCE:
      * dist1 rows come from a per-tile free-axis max,
      * dist2 comes from an elementwise column-accumulator max across the 32
        row-tiles; the partition-axis reduction of the accumulator and the
        combination of the two N-halves happen on the host (the [128, 8192]
        fp16 accumulator is DMA'd out in chunks; this is ~free on-device,
        whereas the gpsimd partition_all_reduce it replaces ran 27us and
        contended with the DVE for the shared SBUF port).
  - Per 128-row tile, 16 matmuls fill 4 PSUM groups of [128, 2048].  The
    ScalarEngine drains each group to SBUF with an fp32->fp16 downcast
    (fp16 keeps min errors ~2^-11; tolerance is 2e-2).  The VectorEngine then
    consumes each staged tile with 2x-mode tensor_tensor ops only - measured:
    TT fp16 runs at 2 elem/cyc while every reduce-shaped op (tensor_reduce,
    Max8, Pool, tensor_scalar+accum) runs at 1 elem/cyc and
    tensor_tensor_reduce crashes the exec unit:
      * row-max: one TT-max L1 fold per tile pair plus one quad-batched
        L2048 fold; the [128, 4, 2048] level is then DMA'd to DRAM and the
        host finishes the last 2048->1 max.  The tree's lower levels cost
        ~5us/quad of low-intensity DVE time but are only 2MB of data that
        hides in the ~35us quad period on idle DMA queues (-20us measured,
        bit-identical results).
      * column accumulator: one TT max per tile (4.3us).
  - Pipeline shaping (the kernel is DVE-bound; end ~= DVE_start + DVE_busy +
    ramp gaps + tail; note ACT at ~7.7us/tile is only slightly below the
    DVE's ~8.8us/tile, so the tree work must stay interleaved at quad
    granularity - batching it per-octet makes the DVE overrun the ACT
    mid-stream and stall ~2.7us per octet, measured):
      * input DMAs are split (first lhsT block + first rhs 512 first) so
        the first matmul starts as early as the DMA infra allows (~11us);
      * PSUM groups run in order g0,g2,g1,g3; tiles 0-3 run per-group
        colacc updates and split L1 halves so the DVE follows the PE's slow
        (mid p-state, 464ns/matmul) ramp cadence instead of waiting for
        whole tiles;
      * steady-state L1 folds are merged across tile pairs ([128, 2, 4096]
        APs over a shared stage buffer) to cut per-op overhead;
      * the ~10us tail is runtime-fixed (end barrier + notification-queue
        DMAs);
      * in the LAST quad the L1 folds are deferred until after the final
        colacc update so the colacc DMA-out overlaps the deferred tree work.
"""

import numpy as np
import ml_dtypes

import concourse.bacc as bacc
import concourse.tile as tile
import concourse.mybir as mybir
from concourse import bass_utils

B = 4
N = 8192
M = 8192
NCORES = 8
NSH = N // 2          # rows per core
NT = NSH // 128       # 32 row tiles per core
K = 24                # augmented contraction rows
GF = 2048             # PSUM drain group size (4 banks; 2 groups in flight)

BF16 = mybir.dt.bfloat16
F16 = mybir.dt.float16
F32 = mybir.dt.float32
MAX = mybir.AluOpType.max
X = mybir.AxisListType.X

# Group order: L1's first half reads groups {0, 2}, second half {1, 3}.
# Producing g0, g2 first lets tile 0's split DVE ops start after 2 drains.
GORDER = (0, 2, 1, 3)


def build_body(tc, lhsT, rhs, d1t, d2t, repeat=1):
    """Emit the kernel body into TileContext `tc`.

    lhsT: [K, NT*128] bf16 AP  (negated augmented rows of this core's N-half)
    rhs:  [K, M]      bf16 AP  (augmented rows of all of xyz2[b])
    d1t:  [NT/4, 128, 4, 4096] f16 AP out (quad-major partial row-max
          tree; host maxes the last axis; [q, r, c] -> -dist1 of point
          (4q+c)*128 + r)
    d2t:  [128, M] f16 AP out (-dist column accumulator; host max-reduces
          over the partition axis)
    """
    nc = tc.nc
    nj = GF // 512
    with (
        tc.tile_pool(name="inp", bufs=1) as inp_pool,
        tc.tile_pool(name="acc", bufs=1) as acc_pool,
        tc.tile_pool(name="stage", bufs=3) as stage_pool,
        tc.tile_pool(name="scr", bufs=2) as scr_pool,
        tc.tile_pool(name="cacc", bufs=1) as cacc_pool,
        tc.tile_pool(name="psum", bufs=2, space="PSUM") as psum_pool,
    ):
        ls = inp_pool.tile([K, NT * 128], BF16, tag="ls")
        rs = inp_pool.tile([K, M], BF16, tag="rs")
        # Fine-grained first-group pieces: end-neutral when the kernel was
        # DVE-bound, but the pipeline is ACT-bound now and ACT's 6.4us
        # ramp gap (waiting on a single-queue 72KB rs chunk) is on the
        # critical path.  (Tail-side DMA fan-out beyond 8 chunks was
        # separately measured to REGRESS ~10us - queue oversubscription -
        # so only the input side is split.)
        nc.sync.dma_start(rs[:, 0:512], rhs[:, 0:512])
        nc.sync.dma_start(ls[:, 0:128], lhsT[:, 0:128])
        for j in range(512, GF, 512):
            nc.sync.dma_start(rs[:, j:j + 512], rhs[:, j:j + 512])
        nc.sync.dma_start(rs[:, 2 * GF:2 * GF + 1024],
                          rhs[:, 2 * GF:2 * GF + 1024])
        nc.sync.dma_start(rs[:, 2 * GF + 1024:3 * GF],
                          rhs[:, 2 * GF + 1024:3 * GF])
        nc.sync.dma_start(ls[:, 128:NT * 128], lhsT[:, 128:NT * 128])
        nc.sync.dma_start(rs[:, GF:2 * GF], rhs[:, GF:2 * GF])
        nc.sync.dma_start(rs[:, 3 * GF:4 * GF], rhs[:, 3 * GF:4 * GF])

        colacc = cacc_pool.tile([128, M], F16, tag="cacc")

        QT = 4   # row-tiles per scr group
        for _ in range(repeat):
            for ip in range(NT // QT):
                first = ip == 0
                last = ip == NT // QT - 1
                scr = scr_pool.tile([128, QT, 4096], F16, tag="scr")
                deferred = []
                for kp in range(QT // 2):
                    # Two row-tiles share one stage buffer so their L1 folds
                    # merge into a single [128, 2, 4096] op (saves the per-op
                    # init + pipe overhead 16x per core).
                    ramp = ip < 2
                    stp = stage_pool.tile([128, 2, M], F16, tag="st")
                    for u in range(2):
                        k = 2 * kp + u
                        i = QT * ip + k
                        st = stp[:, u, :]
                        for g in GORDER:
                            ps = psum_pool.tile([128, GF], F32, tag="ps")
                            for j in range(nj):
                                nc.tensor.matmul(
                                    ps[:, j * 512:(j + 1) * 512],
                                    ls[:, i * 128:(i + 1) * 128],
                                    rs[:, g * GF + j * 512: g * GF + (j + 1) * 512],
                                    start=True,
                                    stop=True,
                                )
                            nc.scalar.copy(st[:, g * GF:(g + 1) * GF], ps[:])
                        if i == 0:
                            # Fast start: per-group colacc init (4x-mode
                            # copies) + split L1 halves, ready after 2 drains.
                            nc.vector.tensor_copy(colacc[:, 0:GF], st[:, 0:GF])
                            nc.vector.tensor_copy(colacc[:, 2 * GF:3 * GF],
                                                  st[:, 2 * GF:3 * GF])
                            nc.vector.tensor_tensor(scr[:, 0, 0:GF],
                                                    st[:, 0:GF],
                                                    st[:, 2 * GF:3 * GF],
                                                    op=MAX)
                            nc.vector.tensor_copy(colacc[:, GF:2 * GF],
                                                  st[:, GF:2 * GF])
                            nc.vector.tensor_copy(colacc[:, 3 * GF:4 * GF],
                                                  st[:, 3 * GF:4 * GF])
                            nc.vector.tensor_tensor(scr[:, 0, GF:2 * GF],
                                                    st[:, GF:2 * GF],
                                                    st[:, 3 * GF:4 * GF],
                                                    op=MAX)
                            continue
                        if ramp:
                            # Ramp (tiles 1-3): per-group colacc updates and
                            # split L1 halves keep the DVE busy at the PE's
                            # (slower) early cadence instead of waiting for
                            # all four drains.
                            for g, half in ((0, 0), (2, 0), (1, 1), (3, 1)):
                                nc.vector.tensor_tensor(
                                    colacc[:, g * GF:(g + 1) * GF],
                                    st[:, g * GF:(g + 1) * GF],
                                    colacc[:, g * GF:(g + 1) * GF], op=MAX)
                                if g >= 2:
                                    nc.vector.tensor_tensor(
                                        scr[:, k, half * GF:(half + 1) * GF],
                                        st[:, half * GF:(half + 1) * GF],
                                        st[:, (half + 2) * GF:(half + 3) * GF],
                                        op=MAX)
                            continue
                        nc.vector.tensor_tensor(colacc[:], st[:], colacc[:],
                                                op=MAX)
                    if ramp:
                        continue
                    if last:
                        deferred.append((kp, stp))
                    else:
                        nc.vector.tensor_tensor(scr[:, 2 * kp:2 * kp + 2, :4096],
                                                stp[:, :, 0:4096],
                                                stp[:, :, 4096:8192], op=MAX)
                        for k in (2 * kp, 2 * kp + 1):
                            for h in range(4):
                                sl = slice(h * 1024, (h + 1) * 1024)
                                nc.sync.dma_start(d1t[ip][:, k, sl],
                                                  scr[:, k, sl])
                # In the LAST quad the L1 folds run after the final colacc
                # update; each pair's raw L1 output ships to DRAM as soon
                # as it exists (the host finishes the 4096->1 row max).
                # No on-device tree below L1 at all: those levels cost
                # ~9.3us/quad of DVE time but are only 4MB of data that
                # hides in the ~30us quad period on idle DMA queues.
                for kp, stp in deferred:
                    nc.vector.tensor_tensor(scr[:, 2 * kp:2 * kp + 2, :4096],
                                            stp[:, :, 0:4096],
                                            stp[:, :, 4096:8192], op=MAX)
                    for k in (2 * kp, 2 * kp + 1):
                        for h in range(4):
                            sl = slice(h * 1024, (h + 1) * 1024)
                            nc.sync.dma_start(d1t[ip][:, k, sl],
                                              scr[:, k, sl])
                if ip < 2:
                    for k in range(QT):
                        for h in range(4):
                            sl = slice(h * 1024, (h + 1) * 1024)
                            nc.sync.dma_start(d1t[ip][:, k, sl],
                                              scr[:, k, sl])

        # Ship the raw column accumulator; host does the partition reduce.
        # Chunked so the transfer spreads over several DMA queues and hides
        # behind the deferred last-quad tree work.
        NCH = 8
        w = M // NCH
        for c in range(NCH):
            nc.sync.dma_start(d2t[:, c * w:(c + 1) * w],
                              colacc[:, c * w:(c + 1) * w])


def build_kernel(nc, repeat=1):
    lhsT = nc.dram_tensor("lhsT", [K, NT * 128], BF16, kind="ExternalInput")
    rhs = nc.dram_tensor("rhs", [K, M], BF16, kind="ExternalInput")
    # Per-quad dump of the row-max tree at the 2048-wide level; the host
    # finishes the reduction (saves ~40us of 1x/low-intensity DVE work).
    d1t = nc.dram_tensor("d1t", [NT // 4, 128, 4, 4096], F16,
                         kind="ExternalOutput")
    d2t = nc.dram_tensor("d2t", [128, M], F16, kind="ExternalOutput")
    with tile.TileContext(nc) as tc:
        build_body(tc, lhsT.ap(), rhs.ap(), d1t.ap(), d2t.ap(), repeat)
    return nc


def _split3(v):
    """v (fp32) -> three bf16 planes (as fp32) with v ~= h + l + ll."""
    bf = ml_dtypes.bfloat16
    h = v.astype(bf).astype(np.float32)
    l = (v - h).astype(bf).astype(np.float32)
    ll = (v - h - l).astype(bf).astype(np.float32)
    return h, l, ll


def _build_aug(x1, x2):
    """x1 [n,3], x2 [m,3] fp32 -> (L [24,n] bf16, R [24,m] bf16) with
    (L.T @ R)[a,b] ~= -(|x1_a|^2 + |x2_b|^2 - 2 x1_a.x2_b)  (negated)."""
    n = x1.shape[0]
    m = x2.shape[0]
    sq1 = (x1 * x1).sum(-1)
    sq2 = (x2 * x2).sum(-1)
    a = -2.0 * x1
    y = x2
    s1h, s1l, s1ll = _split3(sq1)
    s2h, s2l, s2ll = _split3(sq2)
    ah, al, all_ = _split3(a)
    yh, yl, yll = _split3(y)
    ones_n = np.ones(n, np.float32)
    ones_m = np.ones(m, np.float32)
    Ls = [s1h, s1l, s1ll, ones_n, ones_n, ones_n]
    Rs = [ones_m, ones_m, ones_m, s2h, s2l, s2ll]
    for c in range(3):
        for (L, R) in ((ah, yh), (ah, yl), (ah, yll), (al, yh), (al, yl), (all_, yh)):
            Ls.append(L[:, c])
            Rs.append(R[:, c])
    bf = ml_dtypes.bfloat16
    Lm = np.ascontiguousarray(-np.stack(Ls)).astype(bf)   # negated
    Rm = np.ascontiguousarray(np.stack(Rs)).astype(bf)
    return Lm, Rm


def _make_in_maps(xyz1, xyz2):
    in_maps = []
    for c in range(NCORES):
        b, h = divmod(c, 2)
        L, R = _build_aug(xyz1[b, h * NSH:(h + 1) * NSH], xyz2[b])
        in_maps.append({"lhsT": L, "rhs": R})
    return in_maps


_CACHE = {}


def _get_compiled(repeat=1):
    key = ("nc", repeat)
    if key not in _CACHE:
        nc = bacc.Bacc("TRN2", target_bir_lowering=False, debug=False,
                       num_devices=NCORES)
        build_kernel(nc, repeat=repeat)
        nc.compile()
        _CACHE[key] = nc
    return _CACHE[key]


def _gather(results):
    d1 = np.empty((B, N), np.float32)
    d2 = np.empty((B, M), np.float32)
    for b in range(B):
        r0 = results[2 * b]
        r1 = results[2 * b + 1]
        m0 = r0["d1t"].astype(np.float32).max(axis=3)
        m1 = r1["d1t"].astype(np.float32).max(axis=3)
        d1[b, :NSH] = -m0.transpose(1, 0, 2).reshape(128, NT).T.reshape(-1)
        d1[b, NSH:] = -m1.transpose(1, 0, 2).reshape(128, NT).T.reshape(-1)
        m0 = r0["d2t"].astype(np.float32).max(axis=0)
        m1 = r1["d2t"].astype(np.float32).max(axis=0)
        d2[b] = -np.maximum(m0, m1)
    return d1, d2


def kernel(xyz1, xyz2):
    xyz1 = np.asarray(xyz1, dtype=np.float32)
    xyz2 = np.asarray(xyz2, dtype=np.float32)
    in_maps = _make_in_maps(xyz1, xyz2)
    nc = _get_compiled()
    res = bass_utils.run_bass_kernel_spmd(nc, in_maps, core_ids=list(range(NCORES)))
    return _gather(res.results)


# revision 27
# speedup vs baseline: 1.1901x; 1.1901x over previous
"""Chamfer distance (nn_ChamferDistance) Trainium2 Bass kernel.

Computes, for xyz1/xyz2 of shape (4, 8192, 3) fp32:
    dist[n, m] = |p_n|^2 + |q_m|^2 - 2 p_n.q_m   (per batch)
    dist1 = min over m, dist2 = min over n
Returns (dist1, dist2), each (4, 8192) fp32 — same as the reference.

Strategy (single-pass, negated):
  - The pairwise-distance matrix is produced directly by the TensorEngine via
    an augmented inner product: u_a . v_b = sq(P)[a] + sq(Q)[b] - 2 P_a.Q_b.
    All factors are split into 3 bf16 planes (hi/lo/lolo) so every product the
    PE forms is exact in fp32; dropped cross terms are ~2^-26 relative.  The
    L-side planes are negated on the host so the device computes -dist and
    every reduction becomes a MAX.  Host negates the outputs back.
  - Sharding: 8 cores = 4 batches x 2 halves of N.  Each core computes its
    4096 x 8192 block of -dist ONCE:
      * dist1 rows come from a per-tile free-axis max,
      * dist2 comes from an elementwise column-accumulator max across the 32
        row-tiles; the partition-axis reduction of the accumulator and the
        combination of the two N-halves happen on the host (the [128, 8192]
        fp16 accumulator is DMA'd out in chunks; this is ~free on-device,
        whereas the gpsimd partition_all_reduce it replaces ran 27us and
        contended with the DVE for the shared SBUF port).
  - Per 128-row tile, 16 matmuls fill 4 PSUM groups of [128, 2048].  The
    ScalarEngine drains each group to SBUF with an fp32->fp16 downcast
    (fp16 keeps min errors ~2^-11; tolerance is 2e-2).  The VectorEngine then
    consumes each staged tile with 2x-mode tensor_tensor ops only - measured:
    TT fp16 runs at 2 elem/cyc while every reduce-shaped op (tensor_reduce,
    Max8, Pool, tensor_scalar+accum) runs at 1 elem/cyc and
    tensor_tensor_reduce crashes the exec unit:
      * row-max: one TT-max L1 fold per tile pair plus one quad-batched
        L2048 fold; the [128, 4, 2048] level is then DMA'd to DRAM and the
        host finishes the last 2048->1 max.  The tree's lower levels cost
        ~5us/quad of low-intensity DVE time but are only 2MB of data that
        hides in the ~35us quad period on idle DMA queues (-20us measured,
        bit-identical results).
      * column accumulator: one TT max per tile (4.3us).
  - Pipeline shaping (the kernel is DVE-bound; end ~= DVE_start + DVE_busy +
    ramp gaps + tail; note ACT at ~7.7us/tile is only slightly below the
    DVE's ~8.8us/tile, so the tree work must stay interleaved at quad
    granularity - batching it per-octet makes the DVE overrun the ACT
    mid-stream and stall ~2.7us per octet, measured):
      * input DMAs are split (first lhsT block + first rhs 512 first) so
        the first matmul starts as early as the DMA infra allows (~11us);
      * PSUM groups run in order g0,g2,g1,g3; tiles 0-3 run per-group
        colacc updates and split L1 halves so the DVE follows the PE's slow
        (mid p-state, 464ns/matmul) ramp cadence instead of waiting for
        whole tiles;
      * steady-state L1 folds are merged across tile pairs ([128, 2, 4096]
        APs over a shared stage buffer) to cut per-op overhead;
      * the ~10us tail is runtime-fixed (end barrier + notification-queue
        DMAs);
      * in the LAST quad the L1 folds are deferred until after the final
        colacc update so the colacc DMA-out overlaps the deferred tree work.
"""

import numpy as np
import ml_dtypes

import concourse.bacc as bacc
import concourse.tile as tile
import concourse.mybir as mybir
from concourse import bass_utils

B = 4
N = 8192
M = 8192
NCORES = 8
NSH = N // 2          # rows per core
NT = NSH // 128       # 32 row tiles per core
K = 24                # augmented contraction rows
GF = 2048             # PSUM drain group size (4 banks; 2 groups in flight)

BF16 = mybir.dt.bfloat16
F16 = mybir.dt.float16
F32 = mybir.dt.float32
MAX = mybir.AluOpType.max
X = mybir.AxisListType.X

# Group order: L1's first half reads groups {0, 2}, second half {1, 3}.
# Producing g0, g2 first lets tile 0's split DVE ops start after 2 drains.
GORDER = (0, 2, 1, 3)


def build_body(tc, lhsT, rhs, d1t, d2t, repeat=1):
    """Emit the kernel body into TileContext `tc`.

    lhsT: [K, NT*128] bf16 AP  (negated augmented rows of this core's N-half)
    rhs:  [K, M]      bf16 AP  (augmented rows of all of xyz2[b])
    d1t:  [NT/4, 128, 4, 4096] f16 AP out (quad-major partial row-max
          tree; host maxes the last axis; [q, r, c] -> -dist1 of point
          (4q+c)*128 + r)
    d2t:  [128, M] f16 AP out (-dist column accumulator; host max-reduces
          over the partition axis)
    """
    nc = tc.nc
    nj = GF // 512
    with (
        tc.tile_pool(name="inp", bufs=1) as inp_pool,
        tc.tile_pool(name="acc", bufs=1) as acc_pool,
        tc.tile_pool(name="stage", bufs=3) as stage_pool,
        tc.tile_pool(name="scr", bufs=2) as scr_pool,
        tc.tile_pool(name="cacc", bufs=1) as cacc_pool,
        tc.tile_pool(name="psum", bufs=2, space="PSUM") as psum_pool,
    ):
        ls = inp_pool.tile([K, NT * 128], BF16, tag="ls")
        rs = inp_pool.tile([K, M], BF16, tag="rs")
        # Fine-grained first-group pieces: end-neutral when the kernel was
        # DVE-bound, but the pipeline is ACT-bound now and ACT's 6.4us
        # ramp gap (waiting on a single-queue 72KB rs chunk) is on the
        # critical path.  (Tail-side fan-out beyond 8 chunks separately
        # measured to REGRESS ~10us — queue oversubscription — so only the
        # input side is split.)
        nc.sync.dma_start(rs[:, 0:512], rhs[:, 0:512])
        nc.sync.dma_start(ls[:, 0:128], lhsT[:, 0:128])
        for j in range(512, GF, 512):
            nc.sync.dma_start(rs[:, j:j + 512], rhs[:, j:j + 512])
        nc.sync.dma_start(rs[:, 2 * GF:2 * GF + 1024],
                          rhs[:, 2 * GF:2 * GF + 1024])
        nc.sync.dma_start(rs[:, 2 * GF + 1024:3 * GF],
                          rhs[:, 2 * GF + 1024:3 * GF])
        nc.sync.dma_start(ls[:, 128:NT * 128], lhsT[:, 128:NT * 128])
        nc.sync.dma_start(rs[:, GF:2 * GF], rhs[:, GF:2 * GF])
        nc.sync.dma_start(rs[:, 3 * GF:4 * GF], rhs[:, 3 * GF:4 * GF])

        colacc = cacc_pool.tile([128, M], F16, tag="cacc")

        QT = 4   # row-tiles per scr group
        for _ in range(repeat):
            for ip in range(NT // QT):
                first = ip == 0
                last = ip == NT // QT - 1
                scr = scr_pool.tile([128, QT, 4096], F16, tag="scr")
                deferred = []
                for kp in range(QT // 2):
                    # Two row-tiles share one stage buffer so their L1 folds
                    # merge into a single [128, 2, 4096] op (saves the per-op
                    # init + pipe overhead 16x per core).
                    ramp = ip < 2
                    stp = stage_pool.tile([128, 2, M], F16, tag="st")
                    for u in range(2):
                        k = 2 * kp + u
                        i = QT * ip + k
                        st = stp[:, u, :]
                        for g in GORDER:
                            ps = psum_pool.tile([128, GF], F32, tag="ps")
                            for j in range(nj):
                                nc.tensor.matmul(
                                    ps[:, j * 512:(j + 1) * 512],
                                    ls[:, i * 128:(i + 1) * 128],
                                    rs[:, g * GF + j * 512: g * GF + (j + 1) * 512],
                                    start=True,
                                    stop=True,
                                )
                            nc.scalar.copy(st[:, g * GF:(g + 1) * GF], ps[:])
                        if i == 0:
                            # Fast start: per-group colacc init (4x-mode
                            # copies) + split L1 halves, ready after 2 drains.
                            nc.vector.tensor_copy(colacc[:, 0:GF], st[:, 0:GF])
                            nc.vector.tensor_copy(colacc[:, 2 * GF:3 * GF],
                                                  st[:, 2 * GF:3 * GF])
                            nc.vector.tensor_tensor(scr[:, 0, 0:GF],
                                                    st[:, 0:GF],
                                                    st[:, 2 * GF:3 * GF],
                                                    op=MAX)
                            nc.vector.tensor_copy(colacc[:, GF:2 * GF],
                                                  st[:, GF:2 * GF])
                            nc.vector.tensor_copy(colacc[:, 3 * GF:4 * GF],
                                                  st[:, 3 * GF:4 * GF])
                            nc.vector.tensor_tensor(scr[:, 0, GF:2 * GF],
                                                    st[:, GF:2 * GF],
                                                    st[:, 3 * GF:4 * GF],
                                                    op=MAX)
                            continue
                        if ramp:
                            # Ramp (tiles 1-3): per-group colacc updates and
                            # split L1 halves keep the DVE busy at the PE's
                            # (slower) early cadence instead of waiting for
                            # all four drains.
                            for g, half in ((0, 0), (2, 0), (1, 1), (3, 1)):
                                nc.vector.tensor_tensor(
                                    colacc[:, g * GF:(g + 1) * GF],
                                    st[:, g * GF:(g + 1) * GF],
                                    colacc[:, g * GF:(g + 1) * GF], op=MAX)
                                if g >= 2:
                                    nc.vector.tensor_tensor(
                                        scr[:, k, half * GF:(half + 1) * GF],
                                        st[:, half * GF:(half + 1) * GF],
                                        st[:, (half + 2) * GF:(half + 3) * GF],
                                        op=MAX)
                            continue
                        nc.vector.tensor_tensor(colacc[:], st[:], colacc[:],
                                                op=MAX)
                    if ramp:
                        continue
                    if last:
                        deferred.append((kp, stp))
                    else:
                        nc.vector.tensor_tensor(scr[:, 2 * kp:2 * kp + 2, :4096],
                                                stp[:, :, 0:4096],
                                                stp[:, :, 4096:8192], op=MAX)
                        for k in (2 * kp, 2 * kp + 1):
                            for h in range(4):
                                sl = slice(h * 1024, (h + 1) * 1024)
                                nc.sync.dma_start(d1t[ip][:, k, sl],
                                                  scr[:, k, sl])
                # In the LAST quad the L1 folds run after the final colacc
                # update; each pair's raw L1 output ships to DRAM as soon
                # as it exists (the host finishes the 4096->1 row max).
                # No on-device tree below L1 at all: those levels cost
                # ~9.3us/quad of DVE time but are only 4MB of data that
                # hides in the ~30us quad period on idle DMA queues.
                for kp, stp in deferred:
                    nc.vector.tensor_tensor(scr[:, 2 * kp:2 * kp + 2, :4096],
                                            stp[:, :, 0:4096],
                                            stp[:, :, 4096:8192], op=MAX)
                    for k in (2 * kp, 2 * kp + 1):
                        for h in range(4):
                            sl = slice(h * 1024, (h + 1) * 1024)
                            nc.sync.dma_start(d1t[ip][:, k, sl],
                                              scr[:, k, sl])
                if ip < 2:
                    for k in range(QT):
                        for h in range(4):
                            sl = slice(h * 1024, (h + 1) * 1024)
                            nc.sync.dma_start(d1t[ip][:, k, sl],
                                              scr[:, k, sl])

        # Ship the raw column accumulator; host does the partition reduce.
        # Chunked so the transfer spreads over several DMA queues and hides
        # behind the deferred last-quad tree work.
        NCH = 8
        w = M // NCH
        for c in range(NCH):
            nc.sync.dma_start(d2t[:, c * w:(c + 1) * w],
                              colacc[:, c * w:(c + 1) * w])


def build_kernel(nc, repeat=1):
    lhsT = nc.dram_tensor("lhsT", [K, NT * 128], BF16, kind="ExternalInput")
    rhs = nc.dram_tensor("rhs", [K, M], BF16, kind="ExternalInput")
    # Per-quad dump of the row-max tree at the 2048-wide level; the host
    # finishes the reduction (saves ~40us of 1x/low-intensity DVE work).
    d1t = nc.dram_tensor("d1t", [NT // 4, 128, 4, 4096], F16,
                         kind="ExternalOutput")
    d2t = nc.dram_tensor("d2t", [128, M], F16, kind="ExternalOutput")
    with tile.TileContext(nc) as tc:
        build_body(tc, lhsT.ap(), rhs.ap(), d1t.ap(), d2t.ap(), repeat)
    return nc


def _split3(v):
    """v (fp32) -> three bf16 planes (as fp32) with v ~= h + l + ll."""
    bf = ml_dtypes.bfloat16
    h = v.astype(bf).astype(np.float32)
    l = (v - h).astype(bf).astype(np.float32)
    ll = (v - h - l).astype(bf).astype(np.float32)
    return h, l, ll


def _build_aug(x1, x2):
    """x1 [n,3], x2 [m,3] fp32 -> (L [24,n] bf16, R [24,m] bf16) with
    (L.T @ R)[a,b] ~= -(|x1_a|^2 + |x2_b|^2 - 2 x1_a.x2_b)  (negated)."""
    n = x1.shape[0]
    m = x2.shape[0]
    sq1 = (x1 * x1).sum(-1)
    sq2 = (x2 * x2).sum(-1)
    a = -2.0 * x1
    y = x2
    s1h, s1l, s1ll = _split3(sq1)
    s2h, s2l, s2ll = _split3(sq2)
    ah, al, all_ = _split3(a)
    yh, yl, yll = _split3(y)
    ones_n = np.ones(n, np.float32)
    ones_m = np.ones(m, np.float32)
    Ls = [s1h, s1l, s1ll, ones_n, ones_n, ones_n]
    Rs = [ones_m, ones_m, ones_m, s2h, s2l, s2ll]
    for c in range(3):
        for (L, R) in ((ah, yh), (ah, yl), (ah, yll), (al, yh), (al, yl), (all_, yh)):
            Ls.append(L[:, c])
            Rs.append(R[:, c])
    bf = ml_dtypes.bfloat16
    Lm = np.ascontiguousarray(-np.stack(Ls)).astype(bf)   # negated
    Rm = np.ascontiguousarray(np.stack(Rs)).astype(bf)
    return Lm, Rm


def _make_in_maps(xyz1, xyz2):
    in_maps = []
    for c in range(NCORES):
        b, h = divmod(c, 2)
        L, R = _build_aug(xyz1[b, h * NSH:(h + 1) * NSH], xyz2[b])
        in_maps.append({"lhsT": L, "rhs": R})
    return in_maps


_CACHE = {}


def _get_compiled(repeat=1):
    key = ("nc", repeat)
    if key not in _CACHE:
        nc = bacc.Bacc("TRN2", target_bir_lowering=False, debug=False,
                       num_devices=NCORES)
        build_kernel(nc, repeat=repeat)
        nc.compile()
        _CACHE[key] = nc
    return _CACHE[key]


def _gather(results):
    d1 = np.empty((B, N), np.float32)
    d2 = np.empty((B, M), np.float32)
    for b in range(B):
        r0 = results[2 * b]
        r1 = results[2 * b + 1]
        m0 = r0["d1t"].astype(np.float32).max(axis=3)
        m1 = r1["d1t"].astype(np.float32).max(axis=3)
        d1[b, :NSH] = -m0.transpose(1, 0, 2).reshape(128, NT).T.reshape(-1)
        d1[b, NSH:] = -m1.transpose(1, 0, 2).reshape(128, NT).T.reshape(-1)
        m0 = r0["d2t"].astype(np.float32).max(axis=0)
        m1 = r1["d2t"].astype(np.float32).max(axis=0)
        d2[b] = -np.maximum(m0, m1)
    return d1, d2


def kernel(xyz1, xyz2):
    xyz1 = np.asarray(xyz1, dtype=np.float32)
    xyz2 = np.asarray(xyz2, dtype=np.float32)
    in_maps = _make_in_maps(xyz1, xyz2)
    nc = _get_compiled()
    res = bass_utils.run_bass_kernel_spmd(nc, in_maps, core_ids=list(range(NCORES)))
    return _gather(res.results)


# revision 29
# speedup vs baseline: 1.1950x; 1.0041x over previous
"""Chamfer distance (nn_ChamferDistance) Trainium2 Bass kernel.

Computes, for xyz1/xyz2 of shape (4, 8192, 3) fp32:
    dist[n, m] = |p_n|^2 + |q_m|^2 - 2 p_n.q_m   (per batch)
    dist1 = min over m, dist2 = min over n
Returns (dist1, dist2), each (4, 8192) fp32 — same as the reference.

Strategy (single-pass, negated):
  - The pairwise-distance matrix is produced directly by the TensorEngine via
    an augmented inner product: u_a . v_b = sq(P)[a] + sq(Q)[b] - 2 P_a.Q_b.
    All factors are split into 3 bf16 planes (hi/lo/lolo) so every product the
    PE forms is exact in fp32; dropped cross terms are ~2^-26 relative.  The
    L-side planes are negated on the host so the device computes -dist and
    every reduction becomes a MAX.  Host negates the outputs back.
  - Sharding: 8 cores = 4 batches x 2 halves of N.  Each core computes its
    4096 x 8192 block of -dist ONCE:
      * dist1 rows come from a per-tile free-axis max,
      * dist2 comes from an elementwise column-accumulator max across the 32
        row-tiles; the partition-axis reduction of the accumulator and the
        combination of the two N-halves happen on the host (the [128, 8192]
        fp16 accumulator is DMA'd out in chunks; this is ~free on-device,
        whereas the gpsimd partition_all_reduce it replaces ran 27us and
        contended with the DVE for the shared SBUF port).
  - Per 128-row tile, 16 matmuls fill 4 PSUM groups of [128, 2048].  The
    ScalarEngine drains each group to SBUF with an fp32->fp16 downcast
    (fp16 keeps min errors ~2^-11; tolerance is 2e-2).  The VectorEngine then
    consumes each staged tile with 2x-mode tensor_tensor ops only - measured:
    TT fp16 runs at 2 elem/cyc while every reduce-shaped op (tensor_reduce,
    Max8, Pool, tensor_scalar+accum) runs at 1 elem/cyc and
    tensor_tensor_reduce crashes the exec unit:
      * row-max: one TT-max L1 fold per tile pair plus one quad-batched
        L2048 fold; the [128, 4, 2048] level is then DMA'd to DRAM and the
        host finishes the last 2048->1 max.  The tree's lower levels cost
        ~5us/quad of low-intensity DVE time but are only 2MB of data that
        hides in the ~35us quad period on idle DMA queues (-20us measured,
        bit-identical results).
      * column accumulator: one TT max per tile (4.3us).
  - Pipeline shaping (the kernel is DVE-bound; end ~= DVE_start + DVE_busy +
    ramp gaps + tail; note ACT at ~7.7us/tile is only slightly below the
    DVE's ~8.8us/tile, so the tree work must stay interleaved at quad
    granularity - batching it per-octet makes the DVE overrun the ACT
    mid-stream and stall ~2.7us per octet, measured):
      * input DMAs are split (first lhsT block + first rhs 512 first) so
        the first matmul starts as early as the DMA infra allows (~11us);
      * PSUM groups run in order g0,g2,g1,g3; tiles 0-3 run per-group
        colacc updates and split L1 halves so the DVE follows the PE's slow
        (mid p-state, 464ns/matmul) ramp cadence instead of waiting for
        whole tiles;
      * steady-state L1 folds are merged across tile pairs ([128, 2, 4096]
        APs over a shared stage buffer) to cut per-op overhead;
      * the ~10us tail is runtime-fixed (end barrier + notification-queue
        DMAs);
      * in the LAST quad the L1 folds are deferred until after the final
        colacc update so the colacc DMA-out overlaps the deferred tree work.
"""

import numpy as np
import ml_dtypes

import concourse.bacc as bacc
import concourse.tile as tile
import concourse.mybir as mybir
from concourse import bass_utils

B = 4
N = 8192
M = 8192
NCORES = 8
NSH = N // 2          # rows per core
NT = NSH // 128       # 32 row tiles per core
K = 24                # augmented contraction rows
GF = 2048             # PSUM drain group size (4 banks; 2 groups in flight)

BF16 = mybir.dt.bfloat16
F16 = mybir.dt.float16
F32 = mybir.dt.float32
MAX = mybir.AluOpType.max
X = mybir.AxisListType.X

# Group order: L1's first half reads groups {0, 2}, second half {1, 3}.
# Producing g0, g2 first lets tile 0's split DVE ops start after 2 drains.
GORDER = (0, 2, 1, 3)


def build_body(tc, lhsT, rhs, d1t, d2t, repeat=1):
    """Emit the kernel body into TileContext `tc`.

    lhsT: [K, NT*128] bf16 AP  (negated augmented rows of this core's N-half)
    rhs:  [K, M]      bf16 AP  (augmented rows of all of xyz2[b])
    d1t:  [NT/4, 128, 4, 4096] f16 AP out (quad-major partial row-max
          tree; host maxes the last axis; [q, r, c] -> -dist1 of point
          (4q+c)*128 + r)
    d2t:  [128, M] f16 AP out (-dist column accumulator; host max-reduces
          over the partition axis)
    """
    nc = tc.nc
    nj = GF // 512
    with (
        tc.tile_pool(name="inp", bufs=1) as inp_pool,
        tc.tile_pool(name="acc", bufs=1) as acc_pool,
        tc.tile_pool(name="stage", bufs=3) as stage_pool,
        tc.tile_pool(name="scr", bufs=2) as scr_pool,
        tc.tile_pool(name="cacc", bufs=1) as cacc_pool,
        tc.tile_pool(name="psum", bufs=2, space="PSUM") as psum_pool,
    ):
        ls = inp_pool.tile([K, NT * 128], BF16, tag="ls")
        rs = inp_pool.tile([K, M], BF16, tag="rs")
        # First matmul only needs rs[:, 0:512] + ls[:, 0:128]; issue those
        # two small DMAs first so the PE starts as soon as possible.
        # NOTE: finer-grained first-group DMA splitting (4x512-col pieces)
        # was measured to pull the first drain 1.5us earlier but the end
        # time is unchanged — the ramp is ACT/PE-cadence-bound, so an
        # earlier DVE start just re-exposes the same cadence wait.
        nc.sync.dma_start(rs[:, 0:512], rhs[:, 0:512])
        nc.sync.dma_start(ls[:, 0:128], lhsT[:, 0:128])
        nc.sync.dma_start(ls[:, 128:NT * 128], lhsT[:, 128:NT * 128])
        nc.sync.dma_start(rs[:, 512:GF], rhs[:, 512:GF])
        for g in GORDER[1:]:
            nc.sync.dma_start(rs[:, g * GF:(g + 1) * GF],
                              rhs[:, g * GF:(g + 1) * GF])

        colacc = cacc_pool.tile([128, M], F16, tag="cacc")

        QT = 4   # row-tiles per scr group
        for _ in range(repeat):
            for ip in range(NT // QT):
                first = ip == 0
                last = ip == NT // QT - 1
                scr = scr_pool.tile([128, QT, 4096], F16, tag="scr")
                deferred = []
                for kp in range(QT // 2):
                    # Two row-tiles share one stage buffer so their L1 folds
                    # merge into a single [128, 2, 4096] op (saves the per-op
                    # init + pipe overhead 16x per core).
                    ramp = ip < 2
                    stp = stage_pool.tile([128, 2, M], F16, tag="st")
                    for u in range(2):
                        k = 2 * kp + u
                        i = QT * ip + k
                        st = stp[:, u, :]
                        for g in GORDER:
                            ps = psum_pool.tile([128, GF], F32, tag="ps")
                            for j in range(nj):
                                nc.tensor.matmul(
                                    ps[:, j * 512:(j + 1) * 512],
                                    ls[:, i * 128:(i + 1) * 128],
                                    rs[:, g * GF + j * 512: g * GF + (j + 1) * 512],
                                    start=True,
                                    stop=True,
                                )
                            nc.scalar.copy(st[:, g * GF:(g + 1) * GF], ps[:])
                        if i == 0:
                            # Fast start: per-group colacc init (4x-mode
                            # copies) + split L1 halves, ready after 2 drains.
                            nc.vector.tensor_copy(colacc[:, 0:GF], st[:, 0:GF])
                            nc.vector.tensor_copy(colacc[:, 2 * GF:3 * GF],
                                                  st[:, 2 * GF:3 * GF])
                            nc.vector.tensor_tensor(scr[:, 0, 0:GF],
                                                    st[:, 0:GF],
                                                    st[:, 2 * GF:3 * GF],
                                                    op=MAX)
                            nc.vector.tensor_copy(colacc[:, GF:2 * GF],
                                                  st[:, GF:2 * GF])
                            nc.vector.tensor_copy(colacc[:, 3 * GF:4 * GF],
                                                  st[:, 3 * GF:4 * GF])
                            nc.vector.tensor_tensor(scr[:, 0, GF:2 * GF],
                                                    st[:, GF:2 * GF],
                                                    st[:, 3 * GF:4 * GF],
                                                    op=MAX)
                            continue
                        if ramp:
                            # Ramp (tiles 1-3): per-group colacc updates and
                            # split L1 halves keep the DVE busy at the PE's
                            # (slower) early cadence instead of waiting for
                            # all four drains.
                            for g, half in ((0, 0), (2, 0), (1, 1), (3, 1)):
                                nc.vector.tensor_tensor(
                                    colacc[:, g * GF:(g + 1) * GF],
                                    st[:, g * GF:(g + 1) * GF],
                                    colacc[:, g * GF:(g + 1) * GF], op=MAX)
                                if g >= 2:
                                    nc.vector.tensor_tensor(
                                        scr[:, k, half * GF:(half + 1) * GF],
                                        st[:, half * GF:(half + 1) * GF],
                                        st[:, (half + 2) * GF:(half + 3) * GF],
                                        op=MAX)
                            continue
                        nc.vector.tensor_tensor(colacc[:], st[:], colacc[:],
                                                op=MAX)
                    if ramp:
                        continue
                    if last and kp == QT // 2 - 1:
                        # Only the final pair is deferred: the first pair's
                        # L1+dump runs inline so its 2MB leaves the queues
                        # before the tail window.
                        deferred.append((kp, stp))
                    else:
                        nc.vector.tensor_tensor(scr[:, 2 * kp:2 * kp + 2, :4096],
                                                stp[:, :, 0:4096],
                                                stp[:, :, 4096:8192], op=MAX)
                        for k in (2 * kp, 2 * kp + 1):
                            for h in range(4):
                                sl = slice(h * 1024, (h + 1) * 1024)
                                nc.sync.dma_start(d1t[ip][:, k, sl],
                                                  scr[:, k, sl])
                # In the LAST quad the L1 folds run after the final colacc
                # update; each pair's raw L1 output ships to DRAM as soon
                # as it exists (the host finishes the 4096->1 row max).
                # No on-device tree below L1 at all: those levels cost
                # ~9.3us/quad of DVE time but are only 4MB of data that
                # hides in the ~30us quad period on idle DMA queues.
                for kp, stp in deferred:
                    nc.vector.tensor_tensor(scr[:, 2 * kp:2 * kp + 2, :4096],
                                            stp[:, :, 0:4096],
                                            stp[:, :, 4096:8192], op=MAX)
                    for k in (2 * kp, 2 * kp + 1):
                        for h in range(4):
                            sl = slice(h * 1024, (h + 1) * 1024)
                            nc.sync.dma_start(d1t[ip][:, k, sl],
                                              scr[:, k, sl])
                if ip < 2:
                    for k in range(QT):
                        for h in range(4):
                            sl = slice(h * 1024, (h + 1) * 1024)
                            nc.sync.dma_start(d1t[ip][:, k, sl],
                                              scr[:, k, sl])

        # Ship the raw column accumulator; host does the partition reduce.
        # Chunked so the transfer spreads over several DMA queues and hides
        # behind the deferred last-quad tree work.
        NCH = 8
        w = M // NCH
        for c in range(NCH):
            nc.sync.dma_start(d2t[:, c * w:(c + 1) * w],
                              colacc[:, c * w:(c + 1) * w])


def build_kernel(nc, repeat=1):
    lhsT = nc.dram_tensor("lhsT", [K, NT * 128], BF16, kind="ExternalInput")
    rhs = nc.dram_tensor("rhs", [K, M], BF16, kind="ExternalInput")
    # Per-quad dump of the row-max tree at the 2048-wide level; the host
    # finishes the reduction (saves ~40us of 1x/low-intensity DVE work).
    d1t = nc.dram_tensor("d1t", [NT // 4, 128, 4, 4096], F16,
                         kind="ExternalOutput")
    d2t = nc.dram_tensor("d2t", [128, M], F16, kind="ExternalOutput")
    with tile.TileContext(nc) as tc:
        build_body(tc, lhsT.ap(), rhs.ap(), d1t.ap(), d2t.ap(), repeat)
    return nc


def _split3(v):
    """v (fp32) -> three bf16 planes (as fp32) with v ~= h + l + ll."""
    bf = ml_dtypes.bfloat16
    h = v.astype(bf).astype(np.float32)
    l = (v - h).astype(bf).astype(np.float32)
    ll = (v - h - l).astype(bf).astype(np.float32)
    return h, l, ll


def _build_aug(x1, x2):
    """x1 [n,3], x2 [m,3] fp32 -> (L [24,n] bf16, R [24,m] bf16) with
    (L.T @ R)[a,b] ~= -(|x1_a|^2 + |x2_b|^2 - 2 x1_a.x2_b)  (negated)."""
    n = x1.shape[0]
    m = x2.shape[0]
    sq1 = (x1 * x1).sum(-1)
    sq2 = (x2 * x2).sum(-1)
    a = -2.0 * x1
    y = x2
    s1h, s1l, s1ll = _split3(sq1)
    s2h, s2l, s2ll = _split3(sq2)
    ah, al, all_ = _split3(a)
    yh, yl, yll = _split3(y)
    ones_n = np.ones(n, np.float32)
    ones_m = np.ones(m, np.float32)
    Ls = [s1h, s1l, s1ll, ones_n, ones_n, ones_n]
    Rs = [ones_m, ones_m, ones_m, s2h, s2l, s2ll]
    for c in range(3):
        for (L, R) in ((ah, yh), (ah, yl), (ah, yll), (al, yh), (al, yl), (all_, yh)):
            Ls.append(L[:, c])
            Rs.append(R[:, c])
    bf = ml_dtypes.bfloat16
    Lm = np.ascontiguousarray(-np.stack(Ls)).astype(bf)   # negated
    Rm = np.ascontiguousarray(np.stack(Rs)).astype(bf)
    return Lm, Rm


def _make_in_maps(xyz1, xyz2):
    in_maps = []
    for c in range(NCORES):
        b, h = divmod(c, 2)
        L, R = _build_aug(xyz1[b, h * NSH:(h + 1) * NSH], xyz2[b])
        in_maps.append({"lhsT": L, "rhs": R})
    return in_maps


_CACHE = {}


def _get_compiled(repeat=1):
    key = ("nc", repeat)
    if key not in _CACHE:
        nc = bacc.Bacc("TRN2", target_bir_lowering=False, debug=False,
                       num_devices=NCORES)
        build_kernel(nc, repeat=repeat)
        nc.compile()
        _CACHE[key] = nc
    return _CACHE[key]


def _gather(results):
    d1 = np.empty((B, N), np.float32)
    d2 = np.empty((B, M), np.float32)
    for b in range(B):
        r0 = results[2 * b]
        r1 = results[2 * b + 1]
        m0 = r0["d1t"].astype(np.float32).max(axis=3)
        m1 = r1["d1t"].astype(np.float32).max(axis=3)
        d1[b, :NSH] = -m0.transpose(1, 0, 2).reshape(128, NT).T.reshape(-1)
        d1[b, NSH:] = -m1.transpose(1, 0, 2).reshape(128, NT).T.reshape(-1)
        m0 = r0["d2t"].astype(np.float32).max(axis=0)
        m1 = r1["d2t"].astype(np.float32).max(axis=0)
        d2[b] = -np.maximum(m0, m1)
    return d1, d2


def kernel(xyz1, xyz2):
    xyz1 = np.asarray(xyz1, dtype=np.float32)
    xyz2 = np.asarray(xyz2, dtype=np.float32)
    in_maps = _make_in_maps(xyz1, xyz2)
    nc = _get_compiled()
    res = bass_utils.run_bass_kernel_spmd(nc, in_maps, core_ids=list(range(NCORES)))
    return _gather(res.results)
